# revision 45
# baseline (speedup 1.0000x reference)
"""Multi-head self-attention TRN2 kernel (data-parallel over batch).

Problem: B=8, S=1024, D=384, H=8, per-head full D->D projections,
causal + key-padding mask, softmax, out_linear (H*D)->D, query-mask output.

Sharding: batch b -> NeuronCore b (8 cores, no collectives).

Host-side weight folding (exact, fp32 numpy; biases handled exactly):
  A_h = Wq_h @ Wk_h^T  -> scores_raw = x A_h x^T + 1 (x wr)^T  with
        wr = Wk_h bq_h folded as a bias into TA = x A_h (the bk rank-1
        term and bq.bk constant are per-query-row and softmax-invariant).
        The K projection disappears: the QK matmul contracts x directly
        against TA.
  C_h = Wv_h @ Wo_h    -> out = sum_h attn_h (x C_h) + bo2 with
        bo2 = bo + sum_h bv_h Wo_h (attn rows sum to 1). V projection and
        out projection collapse into one x @ C_h, and the PV matmul emits
        output-space columns directly.

Per-core dataflow (one batch element), all matmul operands bf16 (f32 psum):
  xTbf [e,s] bf16 resident in SBUF.
  For each head h:
    TA[e',s] (T-layout) = A chunkT @ xTbf (+wr bias via ACT identity)
    U[t,0:384] = xTbf chunkT @ C; U[t,384] = 1   (fused softmax denominator)
    per 512-wide s-group, causally live t-chunks, diagonal chunks trimmed
    to their live s-suffix (N = 512,384,256,128):
      scoresT[t,s] psum = xTbf chunk (lhsT) @ TA          (PE)
      att[t,s] = exp(scale*scores + logm[t])  bf16        (ACT; logm = -60
                 for mask-dead keys else 0 -- keeps dead-row colsums
                 normal-range so 1/colsum never overflows)
      diagonal chunks: zero the causally-dead triangle    (GPSIMD affine)
    per 128-wide q-tile qt (true causal granularity):
      psum[s, 0:385] = sum_{tt<=qt} att_ttT @ U_tt        (PE, one psum)
      recip = 1/psum[:,384:385]                           (DVE, per-s)
      out_acc[s,:] += psum[:, :384] * recip               (DVE STT)
  out[s,:] = out_acc * maskq[s]  -> DRAM

No DRAM bounce, no separate colsum matmuls, no mask tiles: the softmax
denominator rides along as U's 385th column, and masking is an ACT bias
plus a GPSIMD affine_select. bf16 rounding keeps rel err ~3e-3 max-normed,
inside the 2e-2 gate.

Perf notes (measured on HW): PE matmul rows/head = 9216 (TA) + 13824 (QK,
56.25% causal-live at 128-chunk granularity = the K=128 floor) + 9216 (U)
+ 13860 (PV) -> ~165us PE busy, >98% dense between first and last matmul.
HW exec ~182us (baseline 331.6us). Startup is DMA-bound: the sync queue is
the only one hot before ~10us, so the head-0 critical chain (xT half 0,
A0, xT half 1, C0) goes there in consumption order while warm matmuls
hold the PE clock ramp; secondary-queue (scalar) DMAs carry everything
not needed before ~15us. SBUF tile placement measurably affects matmul
issue rate (+-10%) -- the pad tile below is a deliberate layout shim.
"""

import os
from contextlib import ExitStack

import numpy as np

B, S, D, H = 8, 1024, 384, 8
P = 128
DC = D // P          # 3 partition chunks of the d/e axes
NQT = S // P         # 8 q/t tiles of 128
G4 = 4               # q-tiles per s-group (s-block = 512)
NG = NQT // G4       # 2 groups
D1 = D + 1           # U carries a ones column for the softmax denominator
INV_SQRT_D = float(1.0 / np.sqrt(np.float32(D), dtype=np.float32))
LOGM_DEAD = -60.0    # exp bias for mask-dead keys: e^(score-60) stays normal

_BUILT = None


def build():
    import concourse.bass as bass
    import concourse.bacc as bacc
    import concourse.tile as tile
    import concourse.mybir as mybir

    f32 = mybir.dt.float32
    bf16 = mybir.dt.bfloat16

    nc = bacc.Bacc("TRN2", target_bir_lowering=False, debug=False)

    xT_d = nc.dram_tensor("xT", [D, S], bf16, kind="ExternalInput")
    a_d = nc.dram_tensor("A", [H, D, D], bf16, kind="ExternalInput")
    c_d = nc.dram_tensor("C", [H, D, D], bf16, kind="ExternalInput")
    wr_d = nc.dram_tensor("wr", [H, D], f32, kind="ExternalInput")
    bo2_d = nc.dram_tensor("bo2", [P, D], f32, kind="ExternalInput")
    logm_d = nc.dram_tensor("logmT", [P, NQT], f32, kind="ExternalInput")
    maskq_d = nc.dram_tensor("maskqT", [P, NQT], f32, kind="ExternalInput")
    out_d = nc.dram_tensor("out", [S, D], f32, kind="ExternalOutput")

    with tile.TileContext(nc) as tc, ExitStack() as ctx:
        consts = ctx.enter_context(tc.tile_pool(name="consts", bufs=1))
        wpool = ctx.enter_context(tc.tile_pool(name="wpool", bufs=2))
        tatp = ctx.enter_context(tc.tile_pool(name="tatp", bufs=2))
        upool = ctx.enter_context(tc.tile_pool(name="upool", bufs=2))
        attp = ctx.enter_context(tc.tile_pool(name="attp", bufs=2))
        small = ctx.enter_context(tc.tile_pool(name="small", bufs=8))
        opool = ctx.enter_context(tc.tile_pool(name="opool", bufs=4))
        ps_w = ctx.enter_context(tc.tile_pool(name="ps_w", bufs=3, space="PSUM"))
        ps_qk = ctx.enter_context(tc.tile_pool(name="ps_qk", bufs=3, space="PSUM"))
        ps_pv = ctx.enter_context(tc.tile_pool(name="ps_pv", bufs=2, space="PSUM"))

        # ---- setup: spread the startup loads over BOTH hardware DMA
        # queues (Sync + Scalar are the two hwdge engines) so the first
        # head's TA matmuls can start as early as possible.
        # The sync queue is the only DMA queue hot from t=0 (secondary
        # queues take ~10us to come up), so the head-0 critical chain
        # goes there in consumption order: xT half 0, A0, xT half 1 (C0
        # follows from the head loop).
        pad = consts.tile([P, 256], mybir.dt.float32, tag="pad")  # layout shim
        xT_sb = consts.tile([P, DC, S + 8], bf16, tag="xT")
        a0_sb = wpool.tile([P, DC, D], bf16, tag="a")
        nc.sync.dma_start(
            out=xT_sb[:, :, 0:512],
            in_=xT_d.ap()[:, 0:512].rearrange("(c p) s -> p c s", p=P),
        )
        nc.sync.dma_start(
            out=a0_sb, in_=a_d.ap()[0].rearrange("(c p) e -> p c e", p=P)
        )
        nc.sync.dma_start(
            out=xT_sb[:, :, 512:1024],
            in_=xT_d.ap()[:, 512:1024].rearrange("(c p) s -> p c s", p=P),
        )

        logm_sb = consts.tile([P, NQT], f32, tag="logm")
        nc.scalar.dma_start(out=logm_sb, in_=logm_d.ap())

        maskq_sb = consts.tile([P, NQT], f32, tag="maskq")
        nc.scalar.dma_start(out=maskq_sb, in_=maskq_d.ap())

        bo2_sb = consts.tile([P, D], f32, tag="bo2")
        nc.scalar.dma_start(out=bo2_sb, in_=bo2_d.ap())

        # ---- PE warm-up under the initial DMA shadow (HAM clock ramp).
        # One psum tile reused start/stop=True each time: no pool cycling,
        # so the warm matmuls run back-to-back.
        warm = consts.tile([P, P], bf16, tag="warm")
        nc.vector.memset(warm, 0.0)
        wz = consts.tile([P, 512], bf16, tag="warmz")
        nc.vector.memset(wz, 0.0)
        ps_warm = ps_w.tile([P, 512], f32, tag="w", name="ps_warm")
        for _ in range(12):
            nc.tensor.matmul(ps_warm, warm, wz, start=True, stop=True)

        # out_acc is never pre-initialized: head 0's STT accumulates onto
        # bo2_sb directly (in1), later heads accumulate onto out_acc.
        out_acc = consts.tile([P, NQT, D], f32, tag="out_acc")

        # ---- per-head pipeline ----
        n_heads = int(os.environ.get("MHA_HEADS", str(H)))
        for h in range(n_heads):
            a_sb = a0_sb if h == 0 else wpool.tile([P, DC, D], bf16, tag="a")
            c_sb = wpool.tile([P, DC, D], bf16, tag="c")
            wr_sb = wpool.tile([P, DC], f32, tag="wr")
            if h > 0:
                nc.scalar.dma_start(
                    out=a_sb, in_=a_d.ap()[h].rearrange("(c p) e -> p c e", p=P)
                )
            nc.sync.dma_start(
                out=c_sb, in_=c_d.ap()[h].rearrange("(c p) e -> p c e", p=P)
            )
            nc.scalar.dma_start(
                out=wr_sb, in_=wr_d.ap()[h].rearrange("(c p) -> p c", p=P)
            )

            tat = tatp.tile([P, DC, S + 8], bf16, tag="tat")
            u_sb = upool.tile([P, NQT, D1], bf16, tag="u")
            att_g = {}

            # TA [e', s] = A^T x^T (+ wr bias), one 512-wide s-half
            def emit_ta(sh):
                for ec in range(DC):
                    ps = ps_w.tile([P, 512], f32, tag="w")
                    for dc in range(DC):
                        nc.tensor.matmul(
                            ps,
                            a_sb[:, dc, ec * P : (ec + 1) * P],
                            xT_sb[:, dc, sh * 512 : (sh + 1) * 512],
                            start=(dc == 0),
                            stop=(dc == DC - 1),
                        )
                    nc.scalar.activation(
                        out=tat[:, ec, sh * 512 : (sh + 1) * 512],
                        in_=ps,
                        func=mybir.ActivationFunctionType.Identity,
                        bias=wr_sb[:, ec : ec + 1],
                    )

            # U [t, 0:384] = x C ; U[t, 384] = 1
            def emit_u():
                nc.vector.memset(u_sb[:, :, D : D + 1], 1.0)
                for tt in range(NQT):
                    ps = ps_w.tile([P, 512], f32, tag="w")
                    for dc in range(DC):
                        nc.tensor.matmul(
                            ps[:, :D],
                            xT_sb[:, dc, tt * P : (tt + 1) * P],
                            c_sb[:, dc, :],
                            start=(dc == 0),
                            stop=(dc == DC - 1),
                        )
                    nc.vector.tensor_copy(out=u_sb[:, tt, :D], in_=ps[:, :D])

            # scoresT -> exp -> (diag-trim) for one 512-wide s-group
            def emit_qk(qg):
                s0 = qg * 512
                ntt = qg * G4 + G4
                att = attp.tile([P, NQT, 520], bf16, tag="att", name=f"att{qg}")
                att_g[qg] = att
                for tt in range(ntt):
                    jl = max(0, tt - qg * G4)  # diagonal s-offset in 128s
                    n = 512 - jl * P
                    ps = ps_qk.tile([P, 512], f32, tag="qk")
                    for ec in range(DC):
                        nc.tensor.matmul(
                            ps[:, :n],
                            xT_sb[:, ec, tt * P : (tt + 1) * P],
                            tat[:, ec, s0 + jl * P : s0 + 512],
                            start=(ec == 0),
                            stop=(ec == DC - 1),
                        )
                    nc.scalar.activation(
                        out=att[:, tt, jl * P : 512],
                        in_=ps[:, :n],
                        func=mybir.ActivationFunctionType.Exp,
                        scale=INV_SQRT_D,
                        bias=logm_sb[:, tt : tt + 1],
                    )
                    if tt >= qg * G4:
                        # zero the causally-dead triangle (s_local < t_local)
                        nc.gpsimd.affine_select(
                            out=att[:, tt, jl * P : 512],
                            in_=att[:, tt, jl * P : 512],
                            compare_op=mybir.AluOpType.is_ge,
                            fill=0.0,
                            base=0,
                            channel_multiplier=-1,
                            pattern=[[1, n]],
                        )

            if h == 0:
                # QK(g0) slots between the TA halves so head 0's PE has
                # work while the second xT half / C0 DMAs are still landing.
                emit_ta(0)
                emit_qk(0)
                emit_ta(1)
                emit_u()
                emit_qk(1)
            else:
                emit_ta(0)
                emit_ta(1)
                emit_u()
                emit_qk(0)
                emit_qk(1)

            # PV + normalize + accumulate, true 128-granularity causal.
            # q-tile order picked so every psum-buffer reuse (ps_pv bufs=2)
            # is covered by a LONG accumulation: the wait for q-tile k's
            # buffer is hidden by the preceding q-tile's matmul time, so
            # the short early q-tiles (1-4 matmuls < the ~790ns DVE drain
            # chain) go first or last, never in the middle.
            for qt in (0, 4, 5, 6, 7, 3, 2, 1):
                    att = att_g[qt // G4]
                    qi = qt % G4
                    ps = ps_pv.tile([P, D1], f32, tag="pv")
                    for tt in range(qt + 1):
                        nc.tensor.matmul(
                            ps,
                            att[:, tt, qi * P : (qi + 1) * P],
                            u_sb[:, tt, :],
                            start=(tt == 0),
                            stop=(tt == qt),
                        )
                    recip = small.tile([P, 1], f32, tag="recip")
                    nc.vector.reciprocal(out=recip, in_=ps[:, D : D + 1])
                    nc.vector.scalar_tensor_tensor(
                        out=out_acc[:, qt, :],
                        in0=ps[:, :D],
                        scalar=recip,
                        in1=bo2_sb if h == 0 else out_acc[:, qt, :],
                        op0=mybir.AluOpType.mult,
                        op1=mybir.AluOpType.add,
                    )
                    if h == n_heads - 1:
                        # final query-mask + store, interleaved with the
                        # last head's PV tail (ACT engine is idle here)
                        st = opool.tile([P, D], f32, tag="store")
                        nc.scalar.activation(
                            out=st,
                            in_=out_acc[:, qt, :],
                            func=mybir.ActivationFunctionType.Copy,
                            scale=maskq_sb[:, qt : qt + 1],
                        )
                        nc.sync.dma_start(
                            out=out_d.ap()[qt * P : (qt + 1) * P, :], in_=st
                        )

    nc.compile()
    return nc


def _in_maps(x, mask, Wq, bq, Wk, bk, Wv, bv, Wo, bo):
    import ml_dtypes

    bf16 = ml_dtypes.bfloat16
    x = np.asarray(x, np.float32)
    Wq = np.asarray(Wq, np.float32)
    Wk = np.asarray(Wk, np.float32)
    Wv = np.asarray(Wv, np.float32)
    Wo = np.asarray(Wo, np.float32).reshape(H, D, D)
    bq = np.asarray(bq, np.float32)
    bv = np.asarray(bv, np.float32)
    bo = np.asarray(bo, np.float32)

    # folded weights (exact fp32 host math)
    A = np.einsum("hde,hfe->hdf", Wq, Wk)            # scores = x A x^T
    C = np.einsum("hde,hef->hdf", Wv, Wo)            # out += attn (x C)
    wr = np.einsum("hef,hf->he", Wk, bq)             # bq rank-1 term
    bo2 = bo + np.einsum("he,hef->f", bv, Wo)        # bv term (attn sums to 1)

    m = np.asarray(mask) != 0
    logm = np.where(m, np.float32(0.0), np.float32(LOGM_DEAD)).astype(np.float32)
    maskq = m.astype(np.float32)

    shared = {
        "A": A.astype(bf16),
        "C": C.astype(bf16),
        "wr": wr,
        "bo2": np.broadcast_to(bo2[None, :], (P, D)).copy(),
    }
    return [
        {
            "xT": np.ascontiguousarray(x[b].T).astype(bf16),
            "logmT": np.ascontiguousarray(logm[b].reshape(NQT, P).T),
            "maskqT": np.ascontiguousarray(maskq[b].reshape(NQT, P).T),
            **shared,
        }
        for b in range(B)
    ]


def run(inputs, trace=False):
    """inputs: dict from setup_inputs(). Returns (out [B,S,D] f32, results)."""
    from concourse.bass_utils import run_bass_kernel_spmd

    global _BUILT
    if _BUILT is None:
        _BUILT = build()
    nc = _BUILT
    in_maps = _in_maps(**inputs)
    res = run_bass_kernel_spmd(nc, in_maps, core_ids=list(range(B)), trace=trace)
    out = np.stack([np.asarray(res.results[b]["out"], np.float32) for b in range(B)])
    return out, res


def kernel(**inputs):
    out, _ = run(inputs, trace=False)
    return out
